# revision 15
# baseline (speedup 1.0000x reference)
"""Trainium2 Bass kernel for nn_Cholesky_from_z.

Reference computation (per batch sample b, n=128):
    s starts at 0 per row i; for column j: col = z[i,j]*sqrt(1-s) below diag,
    sqrt(1-s) on diag, 0 above; s += col^2.
Closed form: 1-s at (row i, col j) = prod_{k<j} (1 - z[i,k]^2), so
    L[i,j] = z[i,j] * prod_{k<j} sqrt(1-z[i,k]^2)   (j < i)
    L[i,i] =          prod_{k<i} sqrt(1-z[i,k]^2)
i.e. an exclusive cumulative product of g = sqrt(1-z^2) along each matrix
row, independent per row and per sample.

Device mapping: each sample's strictly-lower entries are packed row-major
with a 1.0 sentinel appended after each row (the "diagonal slot"), 8256
slots per sample.  Each core gets 256 samples as TWO 128-sample blocks
concatenated along the free dimension (one sentinel column before each
block for the shift lookback), so every partition carries one 16512-slot
stream.  Per [128 x chunk] window:
    u = Square(z window)            (ACT)  shifted z window
    g = Sqrt(-u + 1)                (ACT)  = sqrt(1-z^2), shifted
        -> g = 0 exactly at each row-start slot (previous slot is the 1.0
           sentinel), which marks segment boundaries for free
    q = scan: state = g*state + b   (DVE tensor_tensor_scan = segmented
                                     exclusive cumprod-of-sqrt, carried
                                     across chunks via `initial`; fp16 out)
The segment-reset vector b (1.0 exactly at each row-start slot) is a
constant pattern DMA'd from HBM; both blocks use the same chunk grid so
b slices serve both.  The DVE runs nothing but the scans.  The final
multiply L = z * q happens on the host during the unpack/scatter epilogue
(the host already holds z in f32).

DMA traffic is grouped into COARSE PIECES (multi-chunk spans, ~5-8 KiB
per-partition segments): per-chunk transfers produced ~2 KiB packets which
capped the single hardware DMA queue near 227 GB/s and made it the
bottleneck; coarse pieces run near peak.  All input piece DMAs are issued
up front (z and b interleaved so early chunks land first); q output pieces
are flushed as their last chunk's scan completes.
u_pool has bufs=1 so consecutive ACT ops chain WAR-dependencies, pinning
the Tile scheduler to strict chunk order on the ACT queue.
"""

import sys

if "/opt/trn_rl_repo" not in sys.path:
    sys.path.insert(0, "/opt/trn_rl_repo")

import numpy as np

B = 2048
N = 128
NZ = N * (N - 1) // 2          # 8128 strictly-lower entries
PACKED = NZ + N                # 8256 slots incl. diagonal sentinels
NCORES = 8
B_CORE = B // NCORES           # 256
NBLK = B_CORE // 128           # 2 blocks, fused along the free dim

# per-block chunk grid (both blocks use it; b slices are shared)
CHUNKS_BLK = [512, 832, 1152, 1536, 1664, 1664, 896]     # sums to 8256
T_BLK = [0, 512, 1344, 2496, 4032, 5696, 7360]           # chunk starts
CMAX = max(CHUNKS_BLK)
# piece groupings (indices into CHUNKS_BLK)
ZGROUPS = [[0], [1, 2], [3, 4], [5, 6]]      # input z pieces per block
BGROUPS = [[0], [1], [2], [3], [4, 5, 6]]    # b pieces (mod-8256 space)
QGROUPS = [[0, 1, 2], [3, 4], [5], [6]]      # output q pieces per block

_prog_cache = {}

# --- host-side index maps ---------------------------------------------------
# packed slot order: row i -> [z[i,0..i-1], diag_i]; row-start offset i(i+1)/2
_rows, _cols = np.tril_indices(N, -1)                  # row-major strict lower
_strict_slots = (_rows * (_rows + 1) // 2 + _cols).astype(np.int64)
_diag_slots = (np.arange(N) * (np.arange(N) + 1) // 2 + np.arange(N)).astype(np.int64)
_rowstart_slots = (np.arange(N) * (np.arange(N) + 1) // 2).astype(np.int64)
# position of each packed slot in the dense [128,128] row-major output
_out_pos = np.empty(PACKED, np.int64)
_out_pos[_strict_slots] = _rows * N + _cols
_out_pos[_diag_slots] = np.arange(N) * N + np.arange(N)


def _build_program():
    import concourse.bacc as bacc
    import concourse.mybir as mybir
    from concourse.tile import TileContext

    f32 = mybir.dt.float32
    f16 = mybir.dt.float16
    Alu = mybir.AluOpType
    Act = mybir.ActivationFunctionType

    nc = bacc.Bacc("TRN2", target_bir_lowering=False, debug=False,
                   num_devices=NCORES)
    # [128, 2*(PACKED+1)]: sentinel col + block0 packed + sentinel col + block1
    z2 = nc.dram_tensor("z2", [128, NBLK * (PACKED + 1)], f16,
                        kind="ExternalInput").ap()
    bp = nc.dram_tensor("bp", [128, PACKED], f16,
                        kind="ExternalInput").ap()
    qp = nc.dram_tensor("qp", [128, NBLK * PACKED], f16,
                        kind="ExternalOutput").ap()

    # piece col ranges (block-local z cols: chunk window = [t0, t0+C+1))
    zpieces = []   # (first_col, width)
    for grp in ZGROUPS:
        lo = T_BLK[grp[0]]
        hi = T_BLK[grp[-1]] + CHUNKS_BLK[grp[-1]] + 1
        zpieces.append((lo, hi - lo))
    zpiece_of = {}
    for pi, grp in enumerate(ZGROUPS):
        for ch in grp:
            zpiece_of[ch] = pi
    bpieces = []
    for grp in BGROUPS:
        lo = T_BLK[grp[0]]
        hi = T_BLK[grp[-1]] + CHUNKS_BLK[grp[-1]]
        bpieces.append((lo, hi - lo))
    bpiece_of = {}
    for pi, grp in enumerate(BGROUPS):
        for ch in grp:
            bpiece_of[ch] = pi
    qpieces = []
    for grp in QGROUPS:
        lo = T_BLK[grp[0]]
        hi = T_BLK[grp[-1]] + CHUNKS_BLK[grp[-1]]
        qpieces.append((lo, hi - lo))
    qpiece_of = {}
    for pi, grp in enumerate(QGROUPS):
        for ch in grp:
            qpiece_of[ch] = pi

    with TileContext(nc) as tc:
        with (
            tc.tile_pool(name="zpc", bufs=1) as z_pool,
            tc.tile_pool(name="bpc", bufs=1) as b_pool,
            tc.tile_pool(name="qpc", bufs=1) as q_pool,
            tc.tile_pool(name="up", bufs=1) as u_pool,
            tc.tile_pool(name="gp", bufs=3) as g_pool,
            tc.tile_pool(name="warm", bufs=1) as warm_pool,
        ):
            # Warm the ACT function table before the first DMA lands: a tiny
            # Sqrt of a framework-constant triggers the (combined
            # Square/Sqrt/Copy) table load with no cross-engine dependency.
            wt = warm_pool.tile([128, 2], f32, tag="warm")
            wconst = nc.const_aps.tensor(1.0, (128, 2), f32)
            nc.scalar.activation(wt[:, 0:2], wconst, Act.Sqrt)

            # --- issue every input piece DMA up front (z and b interleaved
            # so the earliest chunks' data lands first) ---
            ztile = {}
            btile = {}
            issue_order = [("z", 0, 0), ("b", 0), ("z", 0, 1), ("b", 1),
                           ("z", 0, 2), ("b", 2), ("b", 3), ("z", 0, 3),
                           ("b", 4), ("z", 1, 0), ("z", 1, 1), ("z", 1, 2),
                           ("z", 1, 3)]
            for kind, *args in issue_order:
                if kind == "z":
                    blk, pi = args
                    lo, w = zpieces[pi]
                    t = z_pool.tile([128, w], f16, tag=f"z{blk}_{pi}", name=f"z{blk}_{pi}")
                    nc.sync.dma_start(
                        out=t[:, 0:w],
                        in_=z2[:, blk * (PACKED + 1) + lo:
                               blk * (PACKED + 1) + lo + w])
                    ztile[(blk, pi)] = t
                else:
                    (pi,) = args
                    lo, w = bpieces[pi]
                    t = b_pool.tile([128, w], f16, tag=f"b{pi}", name=f"b{pi}")
                    nc.sync.dma_start(out=t[:, 0:w], in_=bp[:, lo:lo + w])
                    btile[pi] = t

            # q piece tiles
            qtile = {}
            for blk in range(NBLK):
                for pi, (lo, w) in enumerate(qpieces):
                    qtile[(blk, pi)] = q_pool.tile([128, w], f16,
                                                   tag=f"q{blk}_{pi}",
                                                   name=f"q{blk}_{pi}")

            qprev = None
            for blk in range(NBLK):
                for ch, (C, t0) in enumerate(zip(CHUNKS_BLK, T_BLK)):
                    zpi = zpiece_of[ch]
                    zt = ztile[(blk, zpi)]
                    zoff = t0 - zpieces[zpi][0]

                    u = u_pool.tile([128, CMAX], f32, tag="u")
                    nc.scalar.activation(u[:, 0:C], zt[:, zoff:zoff + C],
                                         Act.Square)

                    # g = sqrt(1 - u)  (shifted, zero at row starts)
                    g = g_pool.tile([128, CMAX], f32, tag="g")
                    nc.scalar.activation(g[:, 0:C], u[:, 0:C], Act.Sqrt,
                                         bias=1.0, scale=-1.0)

                    bpi = bpiece_of[ch]
                    bt = btile[bpi]
                    boff = t0 - bpieces[bpi][0]

                    qpi = qpiece_of[ch]
                    qt = qtile[(blk, qpi)]
                    qoff = t0 - qpieces[qpi][0]

                    init = 1.0 if qprev is None else qprev
                    nc.vector.tensor_tensor_scan(qt[:, qoff:qoff + C],
                                                 g[:, 0:C],
                                                 bt[:, boff:boff + C], init,
                                                 Alu.mult, Alu.add)
                    qprev = qt[:, qoff + C - 1:qoff + C]

                    # flush the q piece once its last chunk is scanned
                    if ch == QGROUPS[qpi][-1]:
                        lo, w = qpieces[qpi]
                        nc.sync.dma_start(
                            out=qp[:, blk * PACKED + lo:blk * PACKED + lo + w],
                            in_=qt[:, 0:w])
    nc.compile()
    return nc


def _get_program():
    if "nc" not in _prog_cache:
        _prog_cache["nc"] = _build_program()
    return _prog_cache["nc"]


def _run(in_maps, **kw):
    from concourse.bass_utils import run_bass_kernel_spmd

    nc = _get_program()
    return run_bass_kernel_spmd(nc, in_maps, list(range(NCORES)), **kw)


def kernel(inputs: np.ndarray, _return_raw=False, **run_kw) -> np.ndarray:
    assert inputs.shape == (B, NZ), inputs.shape
    zvec = np.ascontiguousarray(inputs, dtype=np.float32)

    # pack per sample: leading 1.0 sentinel col + [z..., 1.0 sentinel] rows
    zpk = np.ones((B, PACKED + 1), np.float16)
    zpk[:, 1 + _strict_slots] = zvec.astype(np.float16)

    bpat = np.zeros((128, PACKED), np.float16)
    bpat[:, _rowstart_slots] = 1.0

    in_maps = []
    for c in range(NCORES):
        blocks = [zpk[c * B_CORE + blk * 128:(c * B_CORE) + (blk + 1) * 128]
                  for blk in range(NBLK)]
        in_maps.append({"z2": np.ascontiguousarray(np.concatenate(blocks, axis=1)),
                        "bp": bpat})
    res = _run(in_maps, **run_kw)

    qv = np.empty((B, PACKED), np.float16)
    for c in range(NCORES):
        qcore = res.results[c]["qp"]
        for blk in range(NBLK):
            qv[c * B_CORE + blk * 128:c * B_CORE + (blk + 1) * 128] = \
                qcore[:, blk * PACKED:(blk + 1) * PACKED]

    # epilogue: L = z * q (z kept in f32 on host; diag slots use z == 1)
    zfull = np.ones((B, PACKED), np.float32)
    zfull[:, _strict_slots] = zvec
    lpacked = zfull * qv.astype(np.float32)

    out = np.zeros((B, N * N), np.float32)
    out[:, _out_pos] = lpacked
    out = out.reshape(B, N, N)
    if _return_raw:
        return out, res
    return out


# revision 16
# speedup vs baseline: 1.2136x; 1.2136x over previous
"""Trainium2 Bass kernel for nn_Cholesky_from_z.

Reference computation (per batch sample b, n=128):
    s starts at 0 per row i; for column j: col = z[i,j]*sqrt(1-s) below diag,
    sqrt(1-s) on diag, 0 above; s += col^2.
Closed form: 1-s at (row i, col j) = prod_{k<j} (1 - z[i,k]^2), so
    L[i,j] = z[i,j] * prod_{k<j} sqrt(1-z[i,k]^2)   (j < i)
    L[i,i] =          prod_{k<i} sqrt(1-z[i,k]^2)
i.e. an exclusive cumulative product of g = sqrt(1-z^2) along each matrix
row, independent per row and per sample.

Device mapping: each sample's strictly-lower entries are packed row-major
with a 1.0 sentinel appended after each row (the "diagonal slot"), 8256
slots total, fp16.  One leading 1.0 column is prepended so every chunk can
read one element back for the shift.  Per [128 samples x chunk] tile:
    u = Square(ztA)                 (ACT)  ztA = shifted z window
    g = Sqrt(-u + 1)                (ACT)  = sqrt(1-z^2), shifted
        -> g = 0 exactly at each row-start slot (previous slot is the 1.0
           sentinel), which marks segment boundaries for free
    q = scan: state = g*state + b   (DVE tensor_tensor_scan = segmented
                                     exclusive cumprod-of-sqrt, carried
                                     across chunks via `initial`; fp16 out)
The segment-reset vector b (1.0 exactly at each row-start slot) is a
constant pattern, precomputed on the host and DMA'd from HBM, so the DVE
runs nothing but the scans.  The final multiply L = z * q happens on the
host during the unpack/scatter epilogue (the host already holds z in f32,
which is also more accurate than a device fp16 multiply).  Input and
output travel as fp16 (validated relfro ~4e-4 vs the 2e-2 budget); the
scan input g stays f32 and the scan state is fp32 in HW.
Batch dim (2048) is sharded 256 samples per core across 8 cores; the two
128-sample blocks are interleaved chunk-wise so the per-block scan carry
chains overlap on the engines.
u_pool has bufs=1 so consecutive ACT ops chain WAR-dependencies, pinning
the Tile scheduler to strict chunk order on the ACT queue (its DMA-time
model otherwise hoists a later Square ahead of an earlier Sqrt and stalls
the scan pipeline).
"""

import sys

if "/opt/trn_rl_repo" not in sys.path:
    sys.path.insert(0, "/opt/trn_rl_repo")

import numpy as np

B = 2048
N = 128
NZ = N * (N - 1) // 2          # 8128 strictly-lower entries
PACKED = NZ + N                # 8256 slots incl. diagonal sentinels
NCORES = 8
B_CORE = B // NCORES           # 256
# ramp chunk schedule: small first/last chunks shorten pipeline fill/drain;
# middle chunks capped ~1600 so DMA prefetch stays ahead of ACT
CHUNKS = [768, 1472, 1600, 1600, 1600, 960, 256]   # sums to PACKED (8256)
CHUNK_OFF = [0, 768, 2240, 3840, 5440, 7040, 8000]
CMAX = max(CHUNKS)

# --- host-side index maps ---------------------------------------------------
# packed slot order: row i -> [z[i,0..i-1], diag_i]; row-start offset i(i+1)/2
_rows, _cols = np.tril_indices(N, -1)                  # row-major strict lower
_strict_slots = (_rows * (_rows + 1) // 2 + _cols).astype(np.int64)
_diag_slots = (np.arange(N) * (np.arange(N) + 1) // 2 + np.arange(N)).astype(np.int64)
_rowstart_slots = (np.arange(N) * (np.arange(N) + 1) // 2).astype(np.int64)
# position of each packed slot in the dense [128,128] row-major output
_out_pos = np.empty(PACKED, np.int64)
_out_pos[_strict_slots] = _rows * N + _cols
_out_pos[_diag_slots] = np.arange(N) * N + np.arange(N)

_prog_cache = {}


def _build_program():
    import concourse.bacc as bacc
    import concourse.mybir as mybir
    from concourse.tile import TileContext

    f32 = mybir.dt.float32
    f16 = mybir.dt.float16
    Alu = mybir.AluOpType
    Act = mybir.ActivationFunctionType

    nc = bacc.Bacc("TRN2", target_bir_lowering=False, debug=False,
                   num_devices=NCORES)
    zp = nc.dram_tensor("zp", [B_CORE, PACKED + 1], f16,
                        kind="ExternalInput").ap()
    bp = nc.dram_tensor("bp", [128, PACKED], f16,
                        kind="ExternalInput").ap()
    qp = nc.dram_tensor("qp", [B_CORE, PACKED], f16,
                        kind="ExternalOutput").ap()

    NBLK = B_CORE // 128
    with TileContext(nc) as tc:
        with (
            tc.tile_pool(name="ioA", bufs=4) as ioA_pool,
            tc.tile_pool(name="up", bufs=1) as u_pool,
            tc.tile_pool(name="gp", bufs=3) as g_pool,
            tc.tile_pool(name="qpl", bufs=3) as q_pool,
            tc.tile_pool(name="bpool", bufs=1) as bpool,
            tc.tile_pool(name="warm", bufs=1) as warm_pool,
        ):
            # Warm the ACT function table before the first DMA lands: a tiny
            # Sqrt triggers the (combined Square/Sqrt/Copy) table load so no
            # ACT_TABLE_LOAD sits on the first chunk's critical path.
            wt = warm_pool.tile([128, 4], f32, tag="warm")
            nc.vector.memset(wt[:, 0:4], 0.0)
            nc.scalar.activation(wt[:, 0:2], wt[:, 2:4], Act.Sqrt)

            btiles = {}
            qprev = {}
            for ch, (C, c0) in enumerate(zip(CHUNKS, CHUNK_OFF)):
                for blk in range(NBLK):
                    r0 = blk * 128
                    # shifted window (covers packed[c0-1 .. c0+C-1])
                    ztA = ioA_pool.tile([128, CMAX + 1], f16, tag="ztA")
                    nc.sync.dma_start(out=ztA[:, 0:C + 1],
                                      in_=zp[r0:r0 + 128, c0:c0 + C + 1])

                    # b: constant row-start mask, streamed from HBM once per
                    # chunk and shared by both blocks.
                    if blk == 0:
                        bt = bpool.tile([128, CMAX], f16, tag=f"b{ch}")
                        nc.sync.dma_start(out=bt[:, 0:C],
                                          in_=bp[:, c0:c0 + C])
                        btiles[ch] = bt
                    bt = btiles[ch]

                    u = u_pool.tile([128, CMAX], f32, tag="u")
                    nc.scalar.activation(u[:, 0:C], ztA[:, 0:C], Act.Square)

                    # g = sqrt(1 - u)  (shifted, zero at row starts)
                    g = g_pool.tile([128, CMAX], f32, tag="g")
                    nc.scalar.activation(g[:, 0:C], u[:, 0:C], Act.Sqrt,
                                         bias=1.0, scale=-1.0)

                    q = q_pool.tile([128, CMAX], f16, tag="q")
                    if ch == 0:
                        init = 1.0
                    else:
                        qp_t, qp_c = qprev[blk]
                        init = qp_t[:, qp_c - 1:qp_c]
                    nc.vector.tensor_tensor_scan(q[:, 0:C], g[:, 0:C],
                                                 bt[:, 0:C], init,
                                                 Alu.mult, Alu.add)
                    qprev[blk] = (q, C)

                    nc.sync.dma_start(out=qp[r0:r0 + 128, c0:c0 + C],
                                      in_=q[:, 0:C])
    nc.compile()
    return nc


def _get_program():
    if "nc" not in _prog_cache:
        _prog_cache["nc"] = _build_program()
    return _prog_cache["nc"]


def _run(in_maps, **kw):
    from concourse.bass_utils import run_bass_kernel_spmd

    nc = _get_program()
    return run_bass_kernel_spmd(nc, in_maps, list(range(NCORES)), **kw)


def kernel(inputs: np.ndarray, _return_raw=False, **run_kw) -> np.ndarray:
    assert inputs.shape == (B, NZ), inputs.shape
    zvec = np.ascontiguousarray(inputs, dtype=np.float32)

    # pack: one leading 1.0 column (shift sentinel) + per-row
    # [z..., 1.0 sentinel], fp16
    zp = np.ones((B, PACKED + 1), np.float16)
    zp[:, 1 + _strict_slots] = zvec.astype(np.float16)

    bpat = np.zeros((128, PACKED), np.float16)
    bpat[:, _rowstart_slots] = 1.0

    in_maps = [
        {"zp": np.ascontiguousarray(zp[c * B_CORE:(c + 1) * B_CORE]),
         "bp": bpat}
        for c in range(NCORES)
    ]
    res = _run(in_maps, **run_kw)

    qv = np.empty((B, PACKED), np.float16)
    for c in range(NCORES):
        qv[c * B_CORE:(c + 1) * B_CORE] = res.results[c]["qp"]

    # epilogue: L = z * q (z kept in f32 on host; diag slots use z == 1)
    zfull = np.ones((B, PACKED), np.float32)
    zfull[:, _strict_slots] = zvec
    lpacked = zfull * qv.astype(np.float32)

    out = np.zeros((B, N * N), np.float32)
    out[:, _out_pos] = lpacked
    out = out.reshape(B, N, N)
    if _return_raw:
        return out, res
    return out
